# revision 1
# baseline (speedup 1.0000x reference)
"""BART attention (B=4, S=2048, D=1024, H=16) on 8 Trainium2 NeuronCores.

Sharding: tensor-parallel across heads.  Core c owns heads {2c, 2c+1}, i.e.
projection output dims [128c, 128c+128) of wq/wk/wv and rows [128c, 128c+128)
of wo.  Each core computes its two heads' attention over the full batch and a
partial output projection; the host sums the 8 partial outputs.

Device layout per core (all matmuls in float32r: full PE speed, ~1e-4 rel):
  qT, kT  [128 head-dims, 8192 tokens]   (transposed projections)
  v_comb  [tokens, 130] = [vA(64) | 1 | vB(64) | 1]  (ones col -> softmax sums)
  scoresT [k-tok, q-tok] per (batch, head): softmax denom = extra out row of
  the ones-augmented attn@v matmul; exp on ScalarE with fused 1/8 scale; the
  1/sum normalization is applied after attn@v (flash-attention style).
"""
import numpy as np

import concourse.bass as bass
import concourse.mybir as mybir
import concourse.tile as tile
from concourse.bass_utils import run_bass_kernel_spmd
from concourse.masks import make_identity
from concourse.vector_clock import ScopedClock

F32 = mybir.dt.float32
F32R = mybir.dt.float32r
EXPF = mybir.ActivationFunctionType.Exp

B, S, D = 4, 2048, 1024
T = B * S                      # 8192 tokens
NCORES = 8
P = 128                        # partitions / head-dims per core
DK = 64                        # head dim
KC = D // P                    # 8 contraction chunks for projections
TCH = 512                      # token chunk (projection N / q-chunk)
NTCH = T // TCH                # 16
VW = 2 * DK + 2                # 130: [vA | 1 | vB | 1]

# ---------------------------------------------------------------------------
# walrus in this toolchain encodes at most ONE sync wait per instruction
# (two on EventSemaphore).  Tile emits more.  Legalize by carrying excess
# waits on same-engine NOPs inserted right before the instruction (engines
# execute in order, so this is equivalent), and by splitting the kernel-tail
# drain's global-clock waits across a chain of drains.
# ---------------------------------------------------------------------------
_split_counter = [0]


def _legalize_waits(nc):
    inserted = 0
    for fn in nc.m.functions:
        for bb in fn.blocks:
            new_insts = []
            changed = False
            for inst in bb.instructions:
                si = inst.sync_info
                waits = list(si.on_wait) if si is not None and si.on_wait else []
                cap = 2 if inst.opcode == "EventSemaphore" else 1
                if len(waits) > cap:
                    excess, keep = waits[:-cap], waits[-cap:]
                    for w in excess:
                        _split_counter[0] += 1
                        nop = mybir.InstNoOp(
                            name=f"I-waitsplit-{_split_counter[0]}", ins=[], outs=[]
                        )
                        nop.engine = inst.engine
                        nop.sync_info = mybir.SyncInfo(on_wait=[w], on_update=[])
                        new_insts.append(nop)
                        inserted += 1
                    si.on_wait = keep
                    changed = True
                new_insts.append(inst)
            if changed:
                bb.instructions.clear()
                for i in new_insts:
                    bb.instructions.append(i)
    return inserted


class _TC(tile.TileContext):
    def _drain_and_barrier(self, tick_clock, wait_clock):
        drain_inst = self.nc.sync.drain()
        wait_clock.add_sem_waits(
            drain_inst.ins, ScopedClock({None: tick_clock.global_clock})
        )
        si = drain_inst.ins.sync_info
        waits = list(si.on_wait or []) if si is not None else []
        if len(waits) > 1:
            si.on_wait = [waits[0]]
            for w in waits[1:]:
                d = self.nc.sync.drain()
                dsi = d.ins.sync_info
                if dsi is None:
                    d.ins.sync_info = mybir.SyncInfo(on_wait=[w], on_update=[])
                else:
                    dsi.on_wait = [w]
        self.nc.all_engine_barrier()
        assert self.sems is not None
        popped = self.nc._tile_sem_poison_stack.pop()
        assert popped is self._sem_poison
        self.nc.clear_and_free_semaphores(list(self.sems.allocated().values()))
        self.nc.all_engine_barrier()


# ---------------------------------------------------------------------------
# device program (identical on all 8 cores; only input data differs)
# ---------------------------------------------------------------------------
def _build_nc(repeat=1):
    nc = bass.Bass("TRN2", target_bir_lowering=False, debug=False,
                   num_devices=NCORES)
    xt = nc.dram_tensor("xt", [D, T], F32R, kind="ExternalInput").ap()
    wqm = nc.dram_tensor("wqm", [D, P], F32R, kind="ExternalInput").ap()
    wqb = nc.dram_tensor("wqb", [1, P], F32R, kind="ExternalInput").ap()
    wkm = nc.dram_tensor("wkm", [D, P], F32R, kind="ExternalInput").ap()
    wkb = nc.dram_tensor("wkb", [1, P], F32R, kind="ExternalInput").ap()
    wvm = nc.dram_tensor("wvm", [D, P], F32R, kind="ExternalInput").ap()
    wvb = nc.dram_tensor("wvb", [1, P], F32R, kind="ExternalInput").ap()
    wot = nc.dram_tensor("wo", [P, D], F32R, kind="ExternalInput").ap()
    bot = nc.dram_tensor("bo", [KC, P], F32, kind="ExternalInput").ap()
    yt = nc.dram_tensor("yt", [D, T], F32, kind="ExternalOutput").ap()

    with _TC(nc) as tc, nc.allow_low_precision(
            reason="float32r is 32-bit; PE rounds internally"):
        _emit(nc, tc, xt, wqm, wqb, wkm, wkb, wvm, wvb, wot, bot, yt,
              repeat=repeat)
    n = _legalize_waits(nc)
    return nc, n


def _emit(nc, tc, xt, wqm, wqb, wkm, wkb, wvm, wvb, wot, bot, yt, repeat=1):
    ctxs = []

    def pool(name, bufs, space="SBUF"):
        p = tc.tile_pool(name=name, bufs=bufs, space=space)
        ctxs.append(p)
        return p.__enter__()

    wpool = pool("w", 1)
    persist = pool("persist", 1)
    xpool = pool("x", 2)
    scrpool = pool("scr", 2)
    epool = pool("e", 3)
    orawpool = pool("oraw", 2)
    sumpool = pool("sums", 2)
    stgpool = pool("stg", 2)
    ystpool = pool("yst", 2)
    spool = pool("ps_s", 2, space="PSUM")     # [128,1024] = 2 banks/slot
    opool = pool("ps_o", 2, space="PSUM")     # 1 bank/slot
    ypool = pool("ps_y", 2, space="PSUM")     # 1 bank/slot

    # ---- constants / weights (loaded once) ----
    wq_sb = wpool.tile([P, KC, P], F32R)
    wk_sb = wpool.tile([P, KC, P], F32R)
    wv_sb = wpool.tile([P, KC, P], F32R)
    nc.sync.dma_start(wq_sb[:], wqm.rearrange("(k p) d -> p k d", p=P))
    nc.sync.dma_start(wk_sb[:], wkm.rearrange("(k p) d -> p k d", p=P))
    nc.sync.dma_start(wv_sb[:], wvm.rearrange("(k p) d -> p k d", p=P))
    wqb_sb = wpool.tile([1, P], F32R)
    wkb_sb = wpool.tile([1, P], F32R)
    wvb_sb = wpool.tile([1, P], F32R)
    nc.sync.dma_start(wqb_sb[:], wqb[0:1, :])
    nc.sync.dma_start(wkb_sb[:], wkb[0:1, :])
    nc.sync.dma_start(wvb_sb[:], wvb[0:1, :])
    wo_sb = wpool.tile([P, D], F32R)
    nc.sync.dma_start(wo_sb[:], wot[:, :])
    bo_sb = wpool.tile([P, KC], F32)
    nc.sync.dma_start(bo_sb[:], bot.rearrange("m p -> p m"))
    # memset can't write float32r; memset f32 then DVE-copy (which rounds)
    ones_f32 = wpool.tile([P, TCH], F32)
    nc.vector.memset(ones_f32[:], 1.0)
    ones_sb = wpool.tile([1, TCH], F32R)
    nc.vector.tensor_copy(ones_sb[:], ones_f32[0:1, :])
    ident = wpool.tile([P, P], F32)
    make_identity(nc, ident[:])

    # ---- persistent activations ----
    qT = persist.tile([P, T], F32R)
    kT = persist.tile([P, T], F32R)
    v_comb = persist.tile([P, T // P, VW], F32R)    # [tok%128, tok-tile, 130]
    nc.vector.tensor_copy(
        v_comb[:, :, DK:DK + 1],
        ones_f32[:, 0:1].broadcast_to([P, T // P, 1]))
    nc.vector.tensor_copy(
        v_comb[:, :, VW - 1:VW],
        ones_f32[:, 0:1].broadcast_to([P, T // P, 1]))

    NQC = S // TCH                # 4 q-chunks per batch
    NKT = S // P                  # 16 k-tiles per batch

    for b in [b for _ in range(repeat) for b in range(B)]:
        t0 = b * S
        # ================= phase P: q/k/v projections for batch b ==========
        for i in range(S // TCH):
            c0 = t0 + i * TCH
            x_ch = xpool.tile([P, KC, TCH], F32R, tag="x")
            nc.sync.dma_start(
                x_ch[:], xt[:, c0:c0 + TCH].rearrange("(k p) n -> p k n", p=P))
            s_t = spool.tile([P, 2 * TCH], F32, tag="s")
            v_ps = ypool.tile([P, TCH], F32, tag="y")
            for kc in range(KC):
                st = kc == 0
                nc.tensor.matmul(s_t[:, 0:TCH], wq_sb[:, kc, :], x_ch[:, kc, :],
                                 start=st, stop=False)
                nc.tensor.matmul(s_t[:, TCH:2 * TCH], wk_sb[:, kc, :],
                                 x_ch[:, kc, :], start=st, stop=False)
                nc.tensor.matmul(v_ps[:], wv_sb[:, kc, :], x_ch[:, kc, :],
                                 start=st, stop=False)
            nc.tensor.matmul(s_t[:, 0:TCH], wqb_sb[:], ones_sb[:],
                             start=False, stop=True)
            nc.tensor.matmul(s_t[:, TCH:2 * TCH], wkb_sb[:], ones_sb[:],
                             start=False, stop=True)
            nc.tensor.matmul(v_ps[:], wvb_sb[:], ones_sb[:],
                             start=False, stop=True)
            nc.vector.tensor_copy(qT[:, c0:c0 + TCH], s_t[:, 0:TCH])
            nc.vector.tensor_copy(kT[:, c0:c0 + TCH], s_t[:, TCH:2 * TCH])
            v_scr = scrpool.tile([P, TCH], F32, tag="vscr")
            nc.vector.tensor_copy(v_scr[:], v_ps[:])
            for tt in range(TCH // P):
                vt = (c0 // P) + tt
                tr = opool.tile([P, TCH], F32, tag="o")
                nc.tensor.transpose(tr[:, 0:P], v_scr[:, tt * P:(tt + 1) * P],
                                    ident[:])
                nc.vector.tensor_copy(v_comb[:, vt, 0:DK], tr[:, 0:DK])
                nc.vector.tensor_copy(v_comb[:, vt, DK + 1:2 * DK + 1],
                                      tr[:, DK:2 * DK])

        # ================= phase A: attention for batch b ==================
        sums_pp = sumpool.tile([2 * NQC, TCH], F32, tag="sumpp")
        oraw = orawpool.tile([P, S], F32R, tag="oraw")
        for qc in range(NQC):
            q0 = t0 + qc * TCH
            ps_oA = opool.tile([DK + 1, TCH], F32, tag="o")
            ps_oB = opool.tile([DK + 1, TCH], F32, tag="o")
            # software pipeline: attn@v for kc runs one step behind the
            # scores/exp of kc+1 so the PE never serializes behind ACT.
            def attnv(kc, e_t):
                vt = (t0 // P) + kc
                nc.tensor.matmul(ps_oA[:], v_comb[:, vt, 0:DK + 1],
                                 e_t[:, 0:TCH],
                                 start=(kc == 0), stop=(kc == NKT - 1))
                nc.tensor.matmul(ps_oB[:], v_comb[:, vt, DK + 1:VW],
                                 e_t[:, TCH:2 * TCH],
                                 start=(kc == 0), stop=(kc == NKT - 1))

            pending = None
            for kc in range(NKT):
                kt0 = t0 + kc * P
                s_t = spool.tile([P, 2 * TCH], F32, tag="s")
                nc.tensor.matmul(s_t[:, 0:TCH], kT[0:DK, kt0:kt0 + P],
                                 qT[0:DK, q0:q0 + TCH], start=True, stop=True)
                nc.tensor.matmul(s_t[:, TCH:2 * TCH], kT[DK:P, kt0:kt0 + P],
                                 qT[DK:P, q0:q0 + TCH], start=True, stop=True)
                e_t = epool.tile([P, 2 * TCH], F32R, tag="e")
                nc.scalar.activation(e_t[:], s_t[:], EXPF, scale=0.125)
                if pending is not None:
                    attnv(*pending)
                pending = (kc, e_t)
            attnv(*pending)
            # stash softmax denominators (row DK) and raw outputs.
            # DVE writes must start at a 32-aligned partition, so stage each
            # sums row at partition 0 and DMA it to its sums_pp row.
            s_stgA = stgpool.tile([1, TCH], F32, tag="sstg")
            nc.vector.tensor_copy(s_stgA[:], ps_oA[DK:DK + 1, :])
            nc.sync.dma_start(sums_pp[2 * qc:2 * qc + 1, :], s_stgA[:])
            s_stgB = stgpool.tile([1, TCH], F32, tag="sstg")
            nc.vector.tensor_copy(s_stgB[:], ps_oB[DK:DK + 1, :])
            nc.sync.dma_start(sums_pp[2 * qc + 1:2 * qc + 2, :], s_stgB[:])
            nc.vector.tensor_copy(oraw[0:DK, qc * TCH:(qc + 1) * TCH],
                                  ps_oA[0:DK, :])
            nc.vector.tensor_copy(oraw[DK:P, qc * TCH:(qc + 1) * TCH],
                                  ps_oB[0:DK, :])
        # normalization: r = 1/sums, broadcast over 64 partitions, multiply
        recip_pp = sumpool.tile([2 * NQC, TCH], F32R, tag="recip")
        nc.vector.reciprocal(recip_pp[:], sums_pp[:])
        for qc in range(NQC):
            for h in range(2):
                r = 2 * qc + h
                stg = stgpool.tile([1, TCH], F32R, tag="stg")
                nc.sync.dma_start(stg[:], recip_pp[r:r + 1, :])
                bc = spool.tile([P, 2 * TCH], F32, tag="s")
                nc.tensor.matmul(bc[0:DK, 0:TCH], ones_sb[0:1, 0:DK], stg[:],
                                 start=True, stop=True)
                sl = slice(qc * TCH, (qc + 1) * TCH)
                nc.vector.tensor_mul(oraw[h * DK:(h + 1) * DK, sl],
                                     oraw[h * DK:(h + 1) * DK, sl],
                                     bc[0:DK, 0:TCH])

        # ================= phase O: output projection for batch b ==========
        for m in range(KC):
            for qc in range(NQC):
                ps_y = ypool.tile([P, TCH], F32, tag="y")
                nc.tensor.matmul(ps_y[:], wo_sb[:, m * P:(m + 1) * P],
                                 oraw[:, qc * TCH:(qc + 1) * TCH],
                                 start=True, stop=True)
                ys = ystpool.tile([P, TCH], F32, tag="yst")
                nc.vector.tensor_scalar_add(ys[:], ps_y[:], bo_sb[:, m:m + 1])
                nc.sync.dma_start(
                    yt[m * P:(m + 1) * P, t0 + qc * TCH:t0 + (qc + 1) * TCH],
                    ys[:])

    for p in reversed(ctxs):
        p.__exit__(None, None, None)


_CACHED = {}


def _get_nc(repeat=1):
    if repeat not in _CACHED:
        _CACHED[repeat] = _build_nc(repeat=repeat)[0]
    return _CACHED[repeat]


def _make_in_maps(x, wq, bq, wk, bk, wv, bv, wo, bo):
    x = np.asarray(x, np.float32)
    wq, bq = np.asarray(wq, np.float32), np.asarray(bq, np.float32)
    wk, bk = np.asarray(wk, np.float32), np.asarray(bk, np.float32)
    wv, bv = np.asarray(wv, np.float32), np.asarray(bv, np.float32)
    wo, bo = np.asarray(wo, np.float32), np.asarray(bo, np.float32)
    xT = np.ascontiguousarray(x.reshape(T, D).T)
    maps = []
    for c in range(NCORES):
        sl = slice(c * P, (c + 1) * P)
        maps.append({
            "xt": xT,
            "wqm": np.ascontiguousarray(wq[:, sl]),
            "wqb": np.ascontiguousarray(bq[sl])[None, :],
            "wkm": np.ascontiguousarray(wk[:, sl]),
            "wkb": np.ascontiguousarray(bk[sl])[None, :],
            "wvm": np.ascontiguousarray(wv[:, sl]),
            "wvb": np.ascontiguousarray(bv[sl])[None, :],
            "wo": np.ascontiguousarray(wo[sl, :]),
            "bo": (bo if c == 0 else np.zeros_like(bo)).reshape(KC, P).copy(),
        })
    return maps


def kernel(x, wq, bq, wk, bk, wv, bv, wo, bo):
    nc = _get_nc()
    in_maps = _make_in_maps(x, wq, bq, wk, bk, wv, bv, wo, bo)
    res = run_bass_kernel_spmd(nc, in_maps, core_ids=list(range(NCORES)),
                               trace=False)
    yT = res.results[0]["yt"].copy()
    for c in range(1, NCORES):
        yT += res.results[c]["yt"]
    return np.ascontiguousarray(yT.T).reshape(B, S, D)



# revision 13
# speedup vs baseline: 1.4545x; 1.4545x over previous
"""BART attention (B=4, S=2048, D=1024, H=16) on 8 Trainium2 NeuronCores.

Sharding: tensor-parallel across heads.  Core c owns heads {2c, 2c+1}, i.e.
projection output dims [128c, 128c+128) of wq/wk/wv and rows [128c, 128c+128)
of wo.  Each core computes its two heads' attention over the full batch and a
partial output projection; the host sums the 8 partial outputs (f16 partials).

Key idea vs the v1 kernel: the TRN2 tensor engine runs at HALF clock until it
has been continuously busy ~3us (p-state ramp), and any idle gap resets the
ramp.  v1 alternated PE-dense projection phases with ACT-bound attention
phases, so the PE always had gaps and ran at the slow p-state throughout
(~2x loss).  This version emits ONE interleaved instruction stream: the
attention loop (scores -> exp -> attn@v, software-pipelined depth 2) is
padded with "filler" PE work (next batch's projections, previous batch's
output projection) so the PE never stalls, stays ramped, and becomes the
sole bottleneck (~811K PE rows ~= 338us at full clock).

Layout per core (f32r matmuls except the attn@v pair which is bf16):
  qT, kT  [128 head-dims, 8192 tokens]  f32r  (persistent)
  v_comb  [tok%128, tok-tile, 130] bf16 = [vA(64) | 1 | vB(64) | 1]
  PSUM (exactly 8 banks): scores [128,1024]x2 (4), attn-out [65,512]x2 (2),
  shared short-lived ring [128,512]x2 (2) for proj/out-proj/transpose/bcast.
  Softmax: exp on ACT (scale 1/8 fused); denominators ride as the ones-row of
  the attn@v matmul; normalization = reciprocal (DVE) -> partition-broadcast
  (PE matmul vs ones) -> fused multiply during the PSUM->SBUF drain.
"""
import numpy as np

import concourse.bass as bass
import concourse.mybir as mybir
import concourse.tile as tile
from concourse.bass_utils import run_bass_kernel_spmd
from concourse.masks import make_identity
from concourse.vector_clock import ScopedClock

F32 = mybir.dt.float32
F32R = mybir.dt.float32r
BF16 = mybir.dt.bfloat16
F16 = mybir.dt.float16
EXPF = mybir.ActivationFunctionType.Exp

B, S, D = 4, 2048, 1024
T = B * S                      # 8192 tokens
NCORES = 8
P = 128                        # partitions / head-dims per core
DK = 64                        # head dim
KC = D // P                    # 8 contraction chunks for projections
TCH = 512                      # token chunk (projection N / q-chunk)
NQC = S // TCH                 # 4 q-chunks per batch
NKT = S // P                   # 16 k-tiles per batch
VW = 2 * DK + 2                # 130: [vA | 1 | vB | 1]

# estimated PE cost (ns) of one matmul row-block, for filler pacing
MM_NS = 213          # N=512 matmul
TR_NS = 53           # 128-row bf16 transpose
STEP_PE = 854        # scores pair + attnv pair
ACT_NS = 1038        # exp on [128,1024]

# ---------------------------------------------------------------------------
# walrus in this toolchain encodes at most ONE sync wait per instruction
# (two on EventSemaphore).  Tile emits more.  Legalize by carrying excess
# waits on same-engine NOPs inserted right before the instruction (engines
# execute in order, so this is equivalent), and by splitting the kernel-tail
# drain's global-clock waits across a chain of drains.
# ---------------------------------------------------------------------------
_split_counter = [0]


def _legalize_waits(nc):
    inserted = 0
    for fn in nc.m.functions:
        for bb in fn.blocks:
            new_insts = []
            changed = False
            for inst in bb.instructions:
                si = inst.sync_info
                waits = list(si.on_wait) if si is not None and si.on_wait else []
                cap = 2 if inst.opcode == "EventSemaphore" else 1
                if len(waits) > cap:
                    excess, keep = waits[:-cap], waits[-cap:]
                    for w in excess:
                        _split_counter[0] += 1
                        nop = mybir.InstNoOp(
                            name=f"I-waitsplit-{_split_counter[0]}", ins=[], outs=[]
                        )
                        nop.engine = inst.engine
                        nop.sync_info = mybir.SyncInfo(on_wait=[w], on_update=[])
                        new_insts.append(nop)
                        inserted += 1
                    si.on_wait = keep
                    changed = True
                new_insts.append(inst)
            if changed:
                bb.instructions.clear()
                for i in new_insts:
                    bb.instructions.append(i)
    return inserted


class _TC(tile.TileContext):
    def _drain_and_barrier(self, tick_clock, wait_clock):
        drain_inst = self.nc.sync.drain()
        wait_clock.add_sem_waits(
            drain_inst.ins, ScopedClock({None: tick_clock.global_clock})
        )
        si = drain_inst.ins.sync_info
        waits = list(si.on_wait or []) if si is not None else []
        if len(waits) > 1:
            si.on_wait = [waits[0]]
            for w in waits[1:]:
                d = self.nc.sync.drain()
                dsi = d.ins.sync_info
                if dsi is None:
                    d.ins.sync_info = mybir.SyncInfo(on_wait=[w], on_update=[])
                else:
                    dsi.on_wait = [w]
        self.nc.all_engine_barrier()
        assert self.sems is not None
        popped = self.nc._tile_sem_poison_stack.pop()
        assert popped is self._sem_poison
        self.nc.clear_and_free_semaphores(list(self.sems.allocated().values()))
        self.nc.all_engine_barrier()


# ---------------------------------------------------------------------------
# device program (identical on all 8 cores; only input data differs)
# ---------------------------------------------------------------------------
def _build_nc(repeat=1):
    nc = bass.Bass("TRN2", target_bir_lowering=False, debug=False,
                   num_devices=NCORES)
    xt = nc.dram_tensor("xt", [D, T], F32R, kind="ExternalInput").ap()
    wqm = nc.dram_tensor("wqm", [D, P], F32R, kind="ExternalInput").ap()
    wqb = nc.dram_tensor("wqb", [P, 1], F32, kind="ExternalInput").ap()
    wkm = nc.dram_tensor("wkm", [D, P], F32R, kind="ExternalInput").ap()
    wkb = nc.dram_tensor("wkb", [P, 1], F32, kind="ExternalInput").ap()
    wvm = nc.dram_tensor("wvm", [D, P], F32R, kind="ExternalInput").ap()
    wvb = nc.dram_tensor("wvb", [P, 1], F32, kind="ExternalInput").ap()
    wot = nc.dram_tensor("wo", [P, D], F32R, kind="ExternalInput").ap()
    bot = nc.dram_tensor("bo", [KC, P], F32, kind="ExternalInput").ap()
    yt = nc.dram_tensor("yt", [D, T], F16, kind="ExternalOutput").ap()

    with _TC(nc) as tc, nc.allow_low_precision(
            reason="float32r/bf16 matmuls; f16 output partials"):
        _emit(nc, tc, xt, wqm, wqb, wkm, wkb, wvm, wvb, wot, bot, yt,
              repeat=repeat)
    _legalize_waits(nc)
    return nc


def _emit(nc, tc, xt, wqm, wqb, wkm, wkb, wvm, wvb, wot, bot, yt, repeat=1):
    ctxs = []

    def pool(name, bufs, space="SBUF"):
        p = tc.tile_pool(name=name, bufs=bufs, space=space)
        ctxs.append(p)
        return p.__enter__()

    wpool = pool("w", 1)
    persist = pool("persist", 1)
    xpool = pool("x", 3)
    vscrpool = pool("vscr", 2)
    epool = pool("e", 4)
    orawpool = pool("oraw", 3)
    sabpool = pool("sab", 2)
    rcpool = pool("rc", 2)
    yspool = pool("ys", 4)
    pspool = pool("ps", 2, space="PSUM")   # tags s(2bk)x2 + o(1bk)x2 + p(1bk)x2

    # ---- constants / weights (loaded once) ----
    wq_sb = wpool.tile([P, KC, P], F32R)
    wk_sb = wpool.tile([P, KC, P], F32R)
    wv_sb = wpool.tile([P, KC, P], F32R)
    nc.sync.dma_start(wq_sb[:], wqm.rearrange("(k p) d -> p k d", p=P))
    nc.sync.dma_start(wk_sb[:], wkm.rearrange("(k p) d -> p k d", p=P))
    nc.sync.dma_start(wv_sb[:], wvm.rearrange("(k p) d -> p k d", p=P))
    wqb_sb = wpool.tile([P, 1], F32)
    wkb_sb = wpool.tile([P, 1], F32)
    wvb_sb = wpool.tile([P, 1], F32)
    nc.sync.dma_start(wqb_sb[:], wqb[:, :])
    nc.sync.dma_start(wkb_sb[:], wkb[:, :])
    nc.sync.dma_start(wvb_sb[:], wvb[:, :])
    wo_sb = wpool.tile([P, D], F32R)
    nc.sync.dma_start(wo_sb[:], wot[:, :])
    bo_sb = wpool.tile([P, KC], F32)
    nc.sync.dma_start(bo_sb[:], bot.rearrange("m p -> p m"))

    ones_f32 = wpool.tile([P, DK], F32)
    nc.vector.memset(ones_f32[:], 1.0)
    onesDK = wpool.tile([1, DK], F32R)     # lhsT for partition-broadcast mm
    nc.vector.tensor_copy(onesDK[:], ones_f32[0:1, :])
    ident_f32 = wpool.tile([P, P], F32)
    make_identity(nc, ident_f32[:])
    ident_bf = wpool.tile([P, P], BF16)
    nc.vector.tensor_copy(ident_bf[:], ident_f32[:])

    # ---- persistent activations ----
    qT = persist.tile([P, T], F32R)
    kT = persist.tile([P, T], F32R)
    v_comb = persist.tile([P, T // P, VW], BF16)   # [tok%128, tile, 130]
    nc.vector.tensor_copy(
        v_comb[:, :, DK:DK + 1],
        ones_f32[:, 0:1].broadcast_to([P, T // P, 1]))
    nc.vector.tensor_copy(
        v_comb[:, :, VW - 1:VW],
        ones_f32[:, 0:1].broadcast_to([P, T // P, 1]))

    G = repeat * B                 # global batch count
    NCHUNK = G * NQC               # global x-chunk count (4 per batch)
    xtiles = {}                    # chunk idx -> live x tile

    def emit_xdma(m):
        if m >= NCHUNK or m in xtiles:
            return
        b = (m // NQC) % B
        c = m % NQC
        c0 = b * S + c * TCH
        x_ch = xpool.tile([P, KC, TCH], F32R, tag="x", name=f"x_{m}")
        nc.sync.dma_start(
            x_ch[:], xt[:, c0:c0 + TCH].rearrange("(k p) n -> p k n", p=P))
        xtiles[m] = x_ch

    # ---------------- projection filler units for global batch g ------------
    # Units are (cost_ns, fn, open_after): open_after=True while a PSUM "p"
    # accumulation group is mid-flight, during which NO other "p"-tag tile
    # may be allocated (side bc matmuls are gated on this) — otherwise the
    # 2-slot ring can hand the accumulating bank to a later alloc, producing
    # an engine-queue deadlock.
    def make_p_units(g):
        b = g % B
        units = []

        def mk_chunk(c):
            m = g * NQC + c
            tok0 = b * S + c * TCH
            cell = {}

            def mk_mm(w_sb, kc0, key):
                # a PAIR of contraction steps per unit: halves the unit count
                # so the filler queue drains faster than it fills (a growing
                # backlog lets O(g-2) units overlap A(g) and deadlock the
                # oraw ring)
                def u():
                    if kc0 == 0:
                        cell[key] = pspool.tile([P, TCH], F32, tag="p",
                                                name=f"ps_{key}_{m}")
                    for kc in (kc0, kc0 + 1):
                        nc.tensor.matmul(cell[key][:], w_sb[:, kc, :],
                                         xtiles[m][:, kc, :],
                                         start=(kc == 0), stop=(kc == KC - 1))
                return u

            def drain_q():
                nc.vector.tensor_scalar_add(
                    qT[:, tok0:tok0 + TCH], cell['q'][:], wqb_sb[:, 0:1])

            def drain_k():
                nc.vector.tensor_scalar_add(
                    kT[:, tok0:tok0 + TCH], cell['k'][:], wkb_sb[:, 0:1])

            def drain_v():
                v_scr = vscrpool.tile([P, TCH], BF16, tag="vscr",
                                      name=f"vscr_{m}")
                nc.vector.tensor_scalar_add(v_scr[:], cell['v'][:],
                                            wvb_sb[:, 0:1])
                cell['vs'] = v_scr

            def mk_tr(tt):
                def u():
                    vt = tok0 // P + tt
                    tr = pspool.tile([P, 2 * TCH], BF16, tag="p",
                                     name=f"tr_{m}_{tt}")
                    nc.tensor.transpose(tr[:, 0:P],
                                        cell['vs'][:, tt * P:(tt + 1) * P],
                                        ident_bf[:])
                    nc.vector.tensor_copy(v_comb[:, vt, 0:DK], tr[:, 0:DK])
                    nc.vector.tensor_copy(v_comb[:, vt, DK + 1:VW - 1],
                                          tr[:, DK:2 * DK])
                return u

            cunits = []
            for key, w_sb, drain in (('q', wq_sb, drain_q),
                                     ('k', wk_sb, drain_k),
                                     ('v', wv_sb, drain_v)):
                for kc0 in range(0, KC, 2):
                    mm = mk_mm(w_sb, kc0, key)
                    if kc0 == KC - 2:
                        def mmd(mm=mm, drain=drain):
                            mm()
                            drain()
                        cunits.append((2 * MM_NS, mmd, False))
                    else:
                        cunits.append((2 * MM_NS, mm, True))
            for tt in range(TCH // P):
                cunits.append((TR_NS, mk_tr(tt), False))
            # x-DMA prefetch: chunk m+2 issued at the end of chunk m's block
            cunits.append((0, lambda m=m: emit_xdma(m + 2), False))
            return cunits

        for c in range(NQC):
            units.extend(mk_chunk(c))
        return units

    # ---------------- out-projection units for (g, qc) ----------------------
    def make_o_units(g, qc, oraw):
        b = g % B
        t0 = b * S
        units = []

        def mk(m):
            def u():
                ps_y = pspool.tile([P, TCH], F32, tag="p", name=f"ps_y_{g}_{qc}_{m}")
                nc.tensor.matmul(ps_y[:], wo_sb[:, m * P:(m + 1) * P],
                                 oraw[:, qc * TCH:(qc + 1) * TCH],
                                 start=True, stop=True)
                ys = yspool.tile([P, TCH], F16, tag="ys", name=f"ys_{g}_{qc}_{m}")
                nc.vector.tensor_scalar_add(ys[:], ps_y[:], bo_sb[:, m:m + 1])
                nc.sync.dma_start(
                    yt[m * P:(m + 1) * P, t0 + qc * TCH:t0 + (qc + 1) * TCH],
                    ys[:])
            return u

        for m in range(KC):
            units.append((MM_NS, mk(m), False))
        return units

    # ---------------- attention + interleaving main loop --------------------
    from collections import deque
    filler = deque()
    sides = deque()                # (due_step, fn) — monotone due order
    pend = deque()                 # (qc_key, kc, fn) pending attnv closures
    step_idx = [0]
    ostate = {}                    # qc_key -> dict(oA,oB,sA,sB,rA,rB,oraw,g,qc)

    def emit_attnv(ent):
        qkey, kc, fn = ent
        fn()
        if kc == NKT - 1:
            st = ostate[qkey]

            def drains(st=st):
                sA = sabpool.tile([DK + 1, TCH], F32R, tag="sa",
                                  name=f"sa_{qkey}")
                nc.vector.tensor_copy(sA[:], st['oA'][:])
                sB = sabpool.tile([DK + 1, TCH], F32R, tag="sb",
                                  name=f"sb_{qkey}")
                nc.vector.tensor_copy(sB[:], st['oB'][:])
                st['sA'], st['sB'] = sA, sB
            drains()
            s0 = step_idx[0]

            def recipA():
                rA = rcpool.tile([1, TCH], F32R, tag="rc", name=f"rA_{qkey}")
                nc.vector.reciprocal(rA[:], st['sA'][DK:DK + 1, :])
                st['rA'] = rA

            def recipB():
                rB = rcpool.tile([1, TCH], F32R, tag="rc", name=f"rB_{qkey}")
                nc.vector.reciprocal(rB[:], st['sB'][DK:DK + 1, :])
                st['rB'] = rB

            # bc matmul + its mult consumer MUST be emitted adjacently:
            # a "p"-ring tile whose consumer lands in the queues several
            # steps later lets intervening p-allocs slot-wait on a future
            # DVE instruction -> engine-queue deadlock.
            def bcmultA():
                bca = pspool.tile([DK, TCH], F32, tag="p", name=f"bcA_{qkey}")
                nc.tensor.matmul(bca[:], onesDK[0:1, :], st['rA'][:],
                                 start=True, stop=True)
                sl = slice(st['qc'] * TCH, (st['qc'] + 1) * TCH)
                nc.vector.tensor_mul(st['oraw'][0:DK, sl], st['sA'][0:DK, :],
                                     bca[:, :])

            def bcmultB():
                bcb = pspool.tile([DK, TCH], F32, tag="p", name=f"bcB_{qkey}")
                nc.tensor.matmul(bcb[:], onesDK[0:1, :], st['rB'][:],
                                 start=True, stop=True)
                sl = slice(st['qc'] * TCH, (st['qc'] + 1) * TCH)
                nc.vector.tensor_mul(st['oraw'][DK:P, sl], st['sB'][0:DK, :],
                                     bcb[:, :])

            def unlock():
                filler.extend(make_o_units(st['g'], st['qc'], st['oraw']))

            sides.append((s0 + 1, recipA))
            sides.append((s0 + 2, recipB))
            sides.append((s0 + 3, bcmultA))
            sides.append((s0 + 4, bcmultB))
            sides.append((s0 + 5, unlock))

    # head: x DMAs for batch 0 + raw projections for batch 0
    for m in range(3):
        emit_xdma(m)
    for cost, fn, _open in make_p_units(0):
        fn()

    debt = [0.0]
    p_open = [False]               # a PSUM "p" accumulation group is open

    def do_step(g, qc, kc, target):
        qkey = (g, qc)
        b = g % B
        t0 = b * S
        q0 = t0 + qc * TCH
        kt0 = t0 + kc * P
        s_t = pspool.tile([P, 2 * TCH], F32, tag="s", name=f"s_{g}_{qc}_{kc}")
        nc.tensor.matmul(s_t[:, 0:TCH], kT[0:DK, kt0:kt0 + P],
                         qT[0:DK, q0:q0 + TCH], start=True, stop=True)
        nc.tensor.matmul(s_t[:, TCH:2 * TCH], kT[DK:P, kt0:kt0 + P],
                         qT[DK:P, q0:q0 + TCH], start=True, stop=True)
        e_t = epool.tile([P, 2 * TCH], BF16, tag="e", name=f"e_{g}_{qc}_{kc}")
        nc.scalar.activation(e_t[:], s_t[:], EXPF, scale=0.125)

        def attnv(qkey=qkey, kc=kc, e_t=e_t):
            st = ostate[qkey]
            if kc == 0:
                st['oA'] = pspool.tile([DK + 1, TCH], F32, tag="o",
                                       name=f"oA_{qkey}")
                st['oB'] = pspool.tile([DK + 1, TCH], F32, tag="o",
                                       name=f"oB_{qkey}")
            vt = (qkey[0] % B) * S // P + kc
            nc.tensor.matmul(st['oA'][:], v_comb[:, vt, 0:DK + 1],
                             e_t[:, 0:TCH],
                             start=(kc == 0), stop=(kc == NKT - 1))
            nc.tensor.matmul(st['oB'][:], v_comb[:, vt, DK + 1:VW],
                             e_t[:, TCH:2 * TCH],
                             start=(kc == 0), stop=(kc == NKT - 1))
        pend.append((qkey, kc, attnv))

        # fillers, paced by debt; at most 2 per step.  If the queue backlog
        # grows anyway, force progress so filler units never lag more than
        # ~a batch behind the attention stream (stale-overlap hazards).
        debt[0] += target - STEP_PE
        nfill = 0
        while filler and nfill < 2 and (debt[0] >= filler[0][0] - 1e-9
                                        or len(filler) > 90):
            cost, fn, opn = filler.popleft()
            fn()
            p_open[0] = opn
            debt[0] -= cost
            nfill += 1
        if debt[0] > 2000:
            debt[0] = 2000

        # due side ops (norm pipeline for a finished q-chunk); the bc side
        # matmuls allocate "p"-ring tiles, so they must wait until no
        # projection accumulation group is open
        while sides and sides[0][0] <= step_idx[0] and not p_open[0]:
            _, fn = sides.popleft()
            fn()

        # software-pipelined attn@v (depth 2)
        if len(pend) > 2:
            emit_attnv(pend.popleft())
        step_idx[0] += 1

    for g in range(G):
        b = g % B
        # stage next batch's projections as filler
        if g + 1 < G:
            filler.extend(make_p_units(g + 1))
        oraw = orawpool.tile([P, S], F32R, tag="oraw", name=f"oraw_{g}")
        for qc in range(NQC):
            ostate[(g, qc)] = {'g': g, 'qc': qc, 'oraw': oraw}
        supply = sum(c for c, _, _ in filler) + 3 * KC * MM_NS
        target = STEP_PE + min(426.0, supply / (NQC * NKT))
        for qc in range(NQC):
            for kc in range(NKT):
                do_step(g, qc, kc, target)

    # tail: flush pipeline, norm ops, remaining fillers.  Sides may allocate
    # "p"-ring tiles, so fully drain the filler queue (closing any open
    # accumulation group) before each side op.
    while pend:
        emit_attnv(pend.popleft())
        step_idx[0] += 1
    while sides or filler:
        while filler:
            _, fn, _opn = filler.popleft()
            fn()
        if sides:
            _, fn = sides.popleft()
            fn()
            step_idx[0] += 1

    for p in reversed(ctxs):
        p.__exit__(None, None, None)


_CACHED = {}


def _get_nc(repeat=1):
    if repeat not in _CACHED:
        _CACHED[repeat] = _build_nc(repeat=repeat)
    return _CACHED[repeat]


def _make_in_maps(x, wq, bq, wk, bk, wv, bv, wo, bo):
    x = np.asarray(x, np.float32)
    wq, bq = np.asarray(wq, np.float32), np.asarray(bq, np.float32)
    wk, bk = np.asarray(wk, np.float32), np.asarray(bk, np.float32)
    wv, bv = np.asarray(wv, np.float32), np.asarray(bv, np.float32)
    wo, bo = np.asarray(wo, np.float32), np.asarray(bo, np.float32)
    xT = np.ascontiguousarray(x.reshape(T, D).T)
    maps = []
    for c in range(NCORES):
        sl = slice(c * P, (c + 1) * P)
        maps.append({
            "xt": xT,
            "wqm": np.ascontiguousarray(wq[:, sl]),
            "wqb": np.ascontiguousarray(bq[sl])[:, None],
            "wkm": np.ascontiguousarray(wk[:, sl]),
            "wkb": np.ascontiguousarray(bk[sl])[:, None],
            "wvm": np.ascontiguousarray(wv[:, sl]),
            "wvb": np.ascontiguousarray(bv[sl])[:, None],
            "wo": np.ascontiguousarray(wo[sl, :]),
            "bo": (bo if c == 0 else np.zeros_like(bo)).reshape(KC, P).copy(),
        })
    return maps


def kernel(x, wq, bq, wk, bk, wv, bv, wo, bo):
    nc = _get_nc()
    in_maps = _make_in_maps(x, wq, bq, wk, bk, wv, bv, wo, bo)
    res = run_bass_kernel_spmd(nc, in_maps, core_ids=list(range(NCORES)),
                               trace=False)
    yT = np.zeros((D, T), np.float32)
    for c in range(NCORES):
        yT += res.results[c]["yt"].astype(np.float32)
    return np.ascontiguousarray(yT.T).reshape(B, S, D)


# revision 15
# speedup vs baseline: 1.6400x; 1.1275x over previous
"""BART attention (B=4, S=2048, D=1024, H=16) on 8 Trainium2 NeuronCores.

Sharding: tensor-parallel across heads.  Core c owns heads {2c, 2c+1}, i.e.
projection output dims [128c, 128c+128) of wq/wk/wv and rows [128c, 128c+128)
of wo.  Each core computes its two heads' attention over the full batch and a
partial output projection; the host sums the 8 partial outputs (f16 partials).

Key idea vs the v1 kernel: the TRN2 tensor engine runs at HALF clock until it
has been continuously busy ~3us (p-state ramp), and any idle gap resets the
ramp.  v1 alternated PE-dense projection phases with ACT-bound attention
phases, so the PE always had gaps and ran at the slow p-state throughout
(~2x loss).  This version emits ONE interleaved instruction stream: the
attention loop (scores -> exp -> attn@v, software-pipelined depth 2) is
padded with "filler" PE work (next batch's projections, previous batch's
output projection) so the PE never stalls, stays ramped, and becomes the
sole bottleneck (~811K PE rows ~= 338us at full clock).

Layout per core (f32r matmuls except the attn@v pair which is bf16):
  qT, kT  [128 head-dims, 8192 tokens]  f32r  (persistent)
  v_comb  [tok%128, tok-tile, 130] bf16 = [vA(64) | 1 | vB(64) | 1]
  PSUM (exactly 8 banks): scores [128,1024]x2 (4), attn-out [65,512]x2 (2),
  shared short-lived ring [128,512]x2 (2) for proj/out-proj/transpose/bcast.
  Softmax: exp on ACT (scale 1/8 fused); denominators ride as the ones-row of
  the attn@v matmul; normalization = reciprocal (DVE) -> partition-broadcast
  (PE matmul vs ones) -> fused multiply during the PSUM->SBUF drain.
"""
import numpy as np

import concourse.bass as bass
import concourse.mybir as mybir
import concourse.tile as tile
from concourse.bass_utils import run_bass_kernel_spmd
from concourse.masks import make_identity
from concourse.vector_clock import ScopedClock

F32 = mybir.dt.float32
F32R = mybir.dt.float32r
BF16 = mybir.dt.bfloat16
F16 = mybir.dt.float16
EXPF = mybir.ActivationFunctionType.Exp

B, S, D = 4, 2048, 1024
T = B * S                      # 8192 tokens
NCORES = 8
P = 128                        # partitions / head-dims per core
DK = 64                        # head dim
KC = D // P                    # 8 contraction chunks for projections
TCH = 512                      # token chunk (projection N / q-chunk)
NQC = S // TCH                 # 4 q-chunks per batch
NKT = S // P                   # 16 k-tiles per batch
VW = 2 * DK + 2                # 130: [vA | 1 | vB | 1]

# estimated PE cost (ns) of one matmul row-block, for filler pacing
MM_NS = 213          # N=512 matmul
TR_NS = 53           # 128-row bf16 transpose
STEP_PE = 854        # scores pair + attnv pair
ACT_NS = 1038        # exp on [128,1024]

# ---------------------------------------------------------------------------
# walrus in this toolchain encodes at most ONE sync wait per instruction
# (two on EventSemaphore).  Tile emits more.  Legalize by carrying excess
# waits on same-engine NOPs inserted right before the instruction (engines
# execute in order, so this is equivalent), and by splitting the kernel-tail
# drain's global-clock waits across a chain of drains.
# ---------------------------------------------------------------------------
_split_counter = [0]


def _legalize_waits(nc):
    inserted = 0
    for fn in nc.m.functions:
        for bb in fn.blocks:
            new_insts = []
            changed = False
            for inst in bb.instructions:
                si = inst.sync_info
                waits = list(si.on_wait) if si is not None and si.on_wait else []
                cap = 2 if inst.opcode == "EventSemaphore" else 1
                if len(waits) > cap:
                    excess, keep = waits[:-cap], waits[-cap:]
                    for w in excess:
                        _split_counter[0] += 1
                        nop = mybir.InstNoOp(
                            name=f"I-waitsplit-{_split_counter[0]}", ins=[], outs=[]
                        )
                        nop.engine = inst.engine
                        nop.sync_info = mybir.SyncInfo(on_wait=[w], on_update=[])
                        new_insts.append(nop)
                        inserted += 1
                    si.on_wait = keep
                    changed = True
                new_insts.append(inst)
            if changed:
                bb.instructions.clear()
                for i in new_insts:
                    bb.instructions.append(i)
    return inserted


class _TC(tile.TileContext):
    def _drain_and_barrier(self, tick_clock, wait_clock):
        drain_inst = self.nc.sync.drain()
        wait_clock.add_sem_waits(
            drain_inst.ins, ScopedClock({None: tick_clock.global_clock})
        )
        si = drain_inst.ins.sync_info
        waits = list(si.on_wait or []) if si is not None else []
        if len(waits) > 1:
            si.on_wait = [waits[0]]
            for w in waits[1:]:
                d = self.nc.sync.drain()
                dsi = d.ins.sync_info
                if dsi is None:
                    d.ins.sync_info = mybir.SyncInfo(on_wait=[w], on_update=[])
                else:
                    dsi.on_wait = [w]
        self.nc.all_engine_barrier()
        assert self.sems is not None
        popped = self.nc._tile_sem_poison_stack.pop()
        assert popped is self._sem_poison
        self.nc.clear_and_free_semaphores(list(self.sems.allocated().values()))
        self.nc.all_engine_barrier()


# ---------------------------------------------------------------------------
# device program (identical on all 8 cores; only input data differs)
# ---------------------------------------------------------------------------
def _build_nc(repeat=1):
    nc = bass.Bass("TRN2", target_bir_lowering=False, debug=False,
                   num_devices=NCORES)
    xt = nc.dram_tensor("xt", [D, T], F32R, kind="ExternalInput").ap()
    wqm = nc.dram_tensor("wqm", [D, P], F32R, kind="ExternalInput").ap()
    wqb = nc.dram_tensor("wqb", [P, 1], F32, kind="ExternalInput").ap()
    wkm = nc.dram_tensor("wkm", [D, P], F32R, kind="ExternalInput").ap()
    wkb = nc.dram_tensor("wkb", [P, 1], F32, kind="ExternalInput").ap()
    wvm = nc.dram_tensor("wvm", [D, P], F32R, kind="ExternalInput").ap()
    wvb = nc.dram_tensor("wvb", [P, 1], F32, kind="ExternalInput").ap()
    wot = nc.dram_tensor("wo", [P, D], F32R, kind="ExternalInput").ap()
    bot = nc.dram_tensor("bo", [KC, P], F32, kind="ExternalInput").ap()
    yt = nc.dram_tensor("yt", [D, T], F16, kind="ExternalOutput").ap()

    with _TC(nc) as tc, nc.allow_low_precision(
            reason="float32r/bf16 matmuls; f16 output partials"):
        _emit(nc, tc, xt, wqm, wqb, wkm, wkb, wvm, wvb, wot, bot, yt,
              repeat=repeat)
    _legalize_waits(nc)
    return nc


def _emit(nc, tc, xt, wqm, wqb, wkm, wkb, wvm, wvb, wot, bot, yt, repeat=1):
    ctxs = []

    def pool(name, bufs, space="SBUF"):
        p = tc.tile_pool(name=name, bufs=bufs, space=space)
        ctxs.append(p)
        return p.__enter__()

    wpool = pool("w", 1)
    persist = pool("persist", 1)
    xpool = pool("x", 3)
    vscrpool = pool("vscr", 2)
    epool = pool("e", 4)
    orawpool = pool("oraw", 3)
    sabpool = pool("sab", 2)
    rcpool = pool("rc", 2)
    yspool = pool("ys", 4)
    pspool = pool("ps", 2, space="PSUM")   # tags s(2bk)x2 + o(1bk)x2 + p(1bk)x2

    # ---- constants / weights (loaded once) ----
    wq_sb = wpool.tile([P, KC, P], F32R)
    wk_sb = wpool.tile([P, KC, P], F32R)
    wv_sb = wpool.tile([P, KC, P], F32R)
    nc.sync.dma_start(wq_sb[:], wqm.rearrange("(k p) d -> p k d", p=P))
    nc.sync.dma_start(wk_sb[:], wkm.rearrange("(k p) d -> p k d", p=P))
    nc.sync.dma_start(wv_sb[:], wvm.rearrange("(k p) d -> p k d", p=P))
    wqb_sb = wpool.tile([P, 1], F32)
    wkb_sb = wpool.tile([P, 1], F32)
    wvb_sb = wpool.tile([P, 1], F32)
    nc.sync.dma_start(wqb_sb[:], wqb[:, :])
    nc.sync.dma_start(wkb_sb[:], wkb[:, :])
    nc.sync.dma_start(wvb_sb[:], wvb[:, :])
    wo_sb = wpool.tile([P, D], F32R)
    nc.sync.dma_start(wo_sb[:], wot[:, :])
    bo_sb = wpool.tile([P, KC], F32)
    nc.sync.dma_start(bo_sb[:], bot.rearrange("m p -> p m"))

    ones_f32 = wpool.tile([P, DK], F32)
    nc.vector.memset(ones_f32[:], 1.0)
    onesDK = wpool.tile([1, DK], F32R)     # lhsT for partition-broadcast mm
    nc.vector.tensor_copy(onesDK[:], ones_f32[0:1, :])
    ident_f32 = wpool.tile([P, P], F32)
    make_identity(nc, ident_f32[:])
    ident_bf = wpool.tile([P, P], BF16)
    nc.vector.tensor_copy(ident_bf[:], ident_f32[:])

    # ---- persistent activations ----
    qT = persist.tile([P, T], F32R)
    kT = persist.tile([P, T], F32R)
    v_comb = persist.tile([P, T // P, VW], BF16)   # [tok%128, tile, 130]
    nc.vector.tensor_copy(
        v_comb[:, :, DK:DK + 1],
        ones_f32[:, 0:1].broadcast_to([P, T // P, 1]))
    nc.vector.tensor_copy(
        v_comb[:, :, VW - 1:VW],
        ones_f32[:, 0:1].broadcast_to([P, T // P, 1]))

    G = repeat * B                 # global batch count
    NCHUNK = G * NQC               # global x-chunk count (4 per batch)
    xtiles = {}                    # chunk idx -> live x tile

    def emit_xdma(m):
        if m >= NCHUNK or m in xtiles:
            return
        b = (m // NQC) % B
        c = m % NQC
        c0 = b * S + c * TCH
        x_ch = xpool.tile([P, KC, TCH], F32R, tag="x", name=f"x_{m}")
        nc.sync.dma_start(
            x_ch[:], xt[:, c0:c0 + TCH].rearrange("(k p) n -> p k n", p=P))
        xtiles[m] = x_ch

    # ---------------- projection filler units for global batch g ------------
    # Units are (cost_ns, fn, open_after): open_after=True while a PSUM "p"
    # accumulation group is mid-flight, during which NO other "p"-tag tile
    # may be allocated (side bc matmuls are gated on this) — otherwise the
    # 2-slot ring can hand the accumulating bank to a later alloc, producing
    # an engine-queue deadlock.
    def make_p_units(g):
        b = g % B
        units = []

        def mk_chunk(c):
            m = g * NQC + c
            tok0 = b * S + c * TCH
            cell = {}

            def mk_mm(w_sb, kc0, key):
                # a PAIR of contraction steps per unit: halves the unit count
                # so the filler queue drains faster than it fills (a growing
                # backlog lets O(g-2) units overlap A(g) and deadlock the
                # oraw ring)
                def u():
                    if kc0 == 0:
                        cell[key] = pspool.tile([P, TCH], F32, tag="p",
                                                name=f"ps_{key}_{m}")
                    for kc in (kc0, kc0 + 1):
                        nc.tensor.matmul(cell[key][:], w_sb[:, kc, :],
                                         xtiles[m][:, kc, :],
                                         start=(kc == 0), stop=(kc == KC - 1))
                return u

            def drain_q():
                nc.vector.tensor_scalar_add(
                    qT[:, tok0:tok0 + TCH], cell['q'][:], wqb_sb[:, 0:1])

            def drain_k():
                nc.vector.tensor_scalar_add(
                    kT[:, tok0:tok0 + TCH], cell['k'][:], wkb_sb[:, 0:1])

            def drain_v():
                v_scr = vscrpool.tile([P, TCH], BF16, tag="vscr",
                                      name=f"vscr_{m}")
                nc.vector.tensor_scalar_add(v_scr[:], cell['v'][:],
                                            wvb_sb[:, 0:1])
                cell['vs'] = v_scr

            def mk_tr(tt):
                def u():
                    vt = tok0 // P + tt
                    tr = pspool.tile([P, 2 * TCH], BF16, tag="p",
                                     name=f"tr_{m}_{tt}")
                    nc.tensor.transpose(tr[:, 0:P],
                                        cell['vs'][:, tt * P:(tt + 1) * P],
                                        ident_bf[:])
                    nc.vector.tensor_copy(v_comb[:, vt, 0:DK], tr[:, 0:DK])
                    nc.vector.tensor_copy(v_comb[:, vt, DK + 1:VW - 1],
                                          tr[:, DK:2 * DK])
                return u

            cunits = []
            for key, w_sb, drain in (('q', wq_sb, drain_q),
                                     ('k', wk_sb, drain_k),
                                     ('v', wv_sb, drain_v)):
                for kc0 in range(0, KC, 2):
                    mm = mk_mm(w_sb, kc0, key)
                    if kc0 == KC - 2:
                        def mmd(mm=mm, drain=drain):
                            mm()
                            drain()
                        cunits.append((2 * MM_NS, mmd, False))
                    else:
                        cunits.append((2 * MM_NS, mm, True))
            for tt in range(TCH // P):
                cunits.append((TR_NS, mk_tr(tt), False))
            # x-DMA prefetch: chunk m+2 issued at the end of chunk m's block
            cunits.append((0, lambda m=m: emit_xdma(m + 2), False))
            return cunits

        for c in range(NQC):
            units.extend(mk_chunk(c))
        return units

    # ---------------- out-projection units for (g, qc) ----------------------
    def make_o_units(g, qc, oraw):
        b = g % B
        t0 = b * S
        units = []

        def mk(m):
            def u():
                ps_y = pspool.tile([P, TCH], F32, tag="p", name=f"ps_y_{g}_{qc}_{m}")
                nc.tensor.matmul(ps_y[:], wo_sb[:, m * P:(m + 1) * P],
                                 oraw[:, qc * TCH:(qc + 1) * TCH],
                                 start=True, stop=True)
                ys = yspool.tile([P, TCH], F16, tag="ys", name=f"ys_{g}_{qc}_{m}")
                # bias-add + f16 convert on the Scalar engine (it has slack;
                # 2-byte output is its fast path) to keep DVE off the drain
                nc.scalar.add(ys[:], ps_y[:], bo_sb[:, m:m + 1])
                nc.sync.dma_start(
                    yt[m * P:(m + 1) * P, t0 + qc * TCH:t0 + (qc + 1) * TCH],
                    ys[:])
            return u

        for m in range(KC):
            units.append((MM_NS, mk(m), False))
        return units

    # ---------------- attention + interleaving main loop --------------------
    from collections import deque
    filler = deque()
    sides = deque()                # (due_step, fn) — monotone due order
    pend = deque()                 # (qc_key, kc, fn) pending attnv closures
    step_idx = [0]
    ostate = {}                    # qc_key -> dict(oA,oB,sA,sB,rA,rB,oraw,g,qc)

    def emit_attnv(ent):
        qkey, kc, fn = ent
        fn()
        if kc == NKT - 1:
            st = ostate[qkey]

            def drains(st=st):
                sA = sabpool.tile([DK + 1, TCH], F32R, tag="sa",
                                  name=f"sa_{qkey}")
                nc.vector.tensor_copy(sA[:], st['oA'][:])
                sB = sabpool.tile([DK + 1, TCH], F32R, tag="sb",
                                  name=f"sb_{qkey}")
                nc.vector.tensor_copy(sB[:], st['oB'][:])
                st['sA'], st['sB'] = sA, sB
            drains()
            s0 = step_idx[0]

            def recipA():
                rA = rcpool.tile([1, TCH], F32R, tag="rc", name=f"rA_{qkey}")
                nc.vector.reciprocal(rA[:], st['sA'][DK:DK + 1, :])
                st['rA'] = rA

            def recipB():
                rB = rcpool.tile([1, TCH], F32R, tag="rc", name=f"rB_{qkey}")
                nc.vector.reciprocal(rB[:], st['sB'][DK:DK + 1, :])
                st['rB'] = rB

            # bc matmul + its mult consumer MUST be emitted adjacently:
            # a "p"-ring tile whose consumer lands in the queues several
            # steps later lets intervening p-allocs slot-wait on a future
            # DVE instruction -> engine-queue deadlock.
            def bcmultA():
                bca = pspool.tile([DK, TCH], F32, tag="p", name=f"bcA_{qkey}")
                nc.tensor.matmul(bca[:], onesDK[0:1, :], st['rA'][:],
                                 start=True, stop=True)
                sl = slice(st['qc'] * TCH, (st['qc'] + 1) * TCH)
                nc.vector.tensor_mul(st['oraw'][0:DK, sl], st['sA'][0:DK, :],
                                     bca[:, :])

            def bcmultB():
                bcb = pspool.tile([DK, TCH], F32, tag="p", name=f"bcB_{qkey}")
                nc.tensor.matmul(bcb[:], onesDK[0:1, :], st['rB'][:],
                                 start=True, stop=True)
                sl = slice(st['qc'] * TCH, (st['qc'] + 1) * TCH)
                nc.vector.tensor_mul(st['oraw'][DK:P, sl], st['sB'][0:DK, :],
                                     bcb[:, :])

            def unlock():
                filler.extend(make_o_units(st['g'], st['qc'], st['oraw']))

            sides.append((s0 + 1, recipA))
            sides.append((s0 + 2, recipB))
            sides.append((s0 + 3, bcmultA))
            sides.append((s0 + 4, bcmultB))
            sides.append((s0 + 5, unlock))

    # head: x DMAs for batch 0 + raw projections for batch 0
    for m in range(3):
        emit_xdma(m)
    for cost, fn, _open in make_p_units(0):
        fn()

    debt = [0.0]
    p_open = [False]               # a PSUM "p" accumulation group is open

    def do_step(g, qc, kc, target):
        qkey = (g, qc)
        b = g % B
        t0 = b * S
        q0 = t0 + qc * TCH
        kt0 = t0 + kc * P
        s_t = pspool.tile([P, 2 * TCH], F32, tag="s", name=f"s_{g}_{qc}_{kc}")
        nc.tensor.matmul(s_t[:, 0:TCH], kT[0:DK, kt0:kt0 + P],
                         qT[0:DK, q0:q0 + TCH], start=True, stop=True)
        nc.tensor.matmul(s_t[:, TCH:2 * TCH], kT[DK:P, kt0:kt0 + P],
                         qT[DK:P, q0:q0 + TCH], start=True, stop=True)
        e_t = epool.tile([P, 2 * TCH], BF16, tag="e", name=f"e_{g}_{qc}_{kc}")
        nc.scalar.activation(e_t[:], s_t[:], EXPF, scale=0.125)

        def attnv(qkey=qkey, kc=kc, e_t=e_t):
            st = ostate[qkey]
            if kc == 0:
                st['oA'] = pspool.tile([DK + 1, TCH], F32, tag="o",
                                       name=f"oA_{qkey}")
                st['oB'] = pspool.tile([DK + 1, TCH], F32, tag="o",
                                       name=f"oB_{qkey}")
            vt = (qkey[0] % B) * S // P + kc
            nc.tensor.matmul(st['oA'][:], v_comb[:, vt, 0:DK + 1],
                             e_t[:, 0:TCH],
                             start=(kc == 0), stop=(kc == NKT - 1))
            nc.tensor.matmul(st['oB'][:], v_comb[:, vt, DK + 1:VW],
                             e_t[:, TCH:2 * TCH],
                             start=(kc == 0), stop=(kc == NKT - 1))
        pend.append((qkey, kc, attnv))

        # fillers: ALWAYS emit one per step if available (uniform PE fill is
        # what keeps the tensor engine's p-state up — a step with no filler
        # leaves a gap that drops the PE clock 2x); a second is debt-paced.
        debt[0] += target - STEP_PE
        nfill = 0
        while filler and nfill < 2 and (nfill == 0
                                        or debt[0] >= filler[0][0] - 1e-9
                                        or len(filler) > 90):
            cost, fn, opn = filler.popleft()
            fn()
            p_open[0] = opn
            debt[0] -= cost
            nfill += 1
        if debt[0] > 2000:
            debt[0] = 2000

        # due side ops (norm pipeline for a finished q-chunk); the bc side
        # matmuls allocate "p"-ring tiles, so they must wait until no
        # projection accumulation group is open
        while sides and sides[0][0] <= step_idx[0] and not p_open[0]:
            _, fn = sides.popleft()
            fn()

        # software-pipelined attn@v (depth 2)
        if len(pend) > 2:
            emit_attnv(pend.popleft())
        step_idx[0] += 1

    for g in range(G):
        b = g % B
        # stage next batch's projections as filler
        if g + 1 < G:
            filler.extend(make_p_units(g + 1))
        oraw = orawpool.tile([P, S], F32R, tag="oraw", name=f"oraw_{g}")
        for qc in range(NQC):
            ostate[(g, qc)] = {'g': g, 'qc': qc, 'oraw': oraw}
        supply = sum(c for c, _, _ in filler) + 3 * KC * MM_NS
        target = STEP_PE + min(426.0, supply / (NQC * NKT))
        for qc in range(NQC):
            for kc in range(NKT):
                do_step(g, qc, kc, target)

    # tail: flush pipeline, norm ops, remaining fillers.  Sides may allocate
    # "p"-ring tiles, so fully drain the filler queue (closing any open
    # accumulation group) before each side op.
    while pend:
        emit_attnv(pend.popleft())
        step_idx[0] += 1
    while sides or filler:
        while filler:
            _, fn, _opn = filler.popleft()
            fn()
        if sides:
            _, fn = sides.popleft()
            fn()
            step_idx[0] += 1

    for p in reversed(ctxs):
        p.__exit__(None, None, None)


_CACHED = {}


def _get_nc(repeat=1):
    if repeat not in _CACHED:
        _CACHED[repeat] = _build_nc(repeat=repeat)
    return _CACHED[repeat]


def _make_in_maps(x, wq, bq, wk, bk, wv, bv, wo, bo):
    x = np.asarray(x, np.float32)
    wq, bq = np.asarray(wq, np.float32), np.asarray(bq, np.float32)
    wk, bk = np.asarray(wk, np.float32), np.asarray(bk, np.float32)
    wv, bv = np.asarray(wv, np.float32), np.asarray(bv, np.float32)
    wo, bo = np.asarray(wo, np.float32), np.asarray(bo, np.float32)
    xT = np.ascontiguousarray(x.reshape(T, D).T)
    maps = []
    for c in range(NCORES):
        sl = slice(c * P, (c + 1) * P)
        maps.append({
            "xt": xT,
            "wqm": np.ascontiguousarray(wq[:, sl]),
            "wqb": np.ascontiguousarray(bq[sl])[:, None],
            "wkm": np.ascontiguousarray(wk[:, sl]),
            "wkb": np.ascontiguousarray(bk[sl])[:, None],
            "wvm": np.ascontiguousarray(wv[:, sl]),
            "wvb": np.ascontiguousarray(bv[sl])[:, None],
            "wo": np.ascontiguousarray(wo[sl, :]),
            "bo": (bo if c == 0 else np.zeros_like(bo)).reshape(KC, P).copy(),
        })
    return maps


def kernel(x, wq, bq, wk, bk, wv, bv, wo, bo):
    nc = _get_nc()
    in_maps = _make_in_maps(x, wq, bq, wk, bk, wv, bv, wo, bo)
    res = run_bass_kernel_spmd(nc, in_maps, core_ids=list(range(NCORES)),
                               trace=False)
    yT = np.zeros((D, T), np.float32)
    for c in range(NCORES):
        yT += res.results[c]["yt"].astype(np.float32)
    return np.ascontiguousarray(yT.T).reshape(B, S, D)


# revision 21
# speedup vs baseline: 1.7286x; 1.0541x over previous
"""BART attention (B=4, S=2048, D=1024, H=16) on 8 Trainium2 NeuronCores.

Sharding: tensor-parallel across heads.  Core c owns heads {2c, 2c+1}, i.e.
projection output dims [128c, 128c+128) of wq/wk/wv and rows [128c, 128c+128)
of wo.  Each core computes its two heads' attention over the full batch and a
partial output projection; the host sums the 8 partial outputs (f16 partials).

Key idea vs the v1 kernel: the TRN2 tensor engine runs at HALF clock until it
has been continuously busy ~3us (p-state ramp), and any idle gap resets the
ramp.  v1 alternated PE-dense projection phases with ACT-bound attention
phases, so the PE always had gaps and ran at the slow p-state throughout
(~2x loss).  This version emits ONE interleaved instruction stream: the
attention loop (scores -> exp -> attn@v, software-pipelined depth 2) is
padded with "filler" PE work (next batch's projections, previous batch's
output projection) so the PE never stalls, stays ramped, and becomes the
sole bottleneck (~811K PE rows ~= 338us at full clock).

Layout per core (f32r matmuls except the attn@v pair which is bf16):
  qT, kT  [128 head-dims, 8192 tokens]  f32r  (persistent)
  v_comb  [tok%128, tok-tile, 130] bf16 = [vA(64) | 1 | vB(64) | 1]
  PSUM (exactly 8 banks): scores [128,1024]x2 (4), attn-out [65,512]x2 (2),
  shared short-lived ring [128,512]x2 (2) for proj/out-proj/transpose/bcast.
  Softmax: exp on ACT (scale 1/8 fused); denominators ride as the ones-row of
  the attn@v matmul; normalization = reciprocal (DVE) -> partition-broadcast
  (PE matmul vs ones) -> fused multiply during the PSUM->SBUF drain.
"""
import numpy as np

import concourse.bass as bass
import concourse.mybir as mybir
import concourse.tile as tile
from concourse.bass_utils import run_bass_kernel_spmd
from concourse.masks import make_identity
from concourse.vector_clock import ScopedClock

F32 = mybir.dt.float32
F32R = mybir.dt.float32r
BF16 = mybir.dt.bfloat16
F16 = mybir.dt.float16
EXPF = mybir.ActivationFunctionType.Exp

B, S, D = 4, 2048, 1024
T = B * S                      # 8192 tokens
NCORES = 8
P = 128                        # partitions / head-dims per core
DK = 64                        # head dim
KC = D // P                    # 8 contraction chunks for projections
TCH = 512                      # token chunk (projection N / q-chunk)
NQC = S // TCH                 # 4 q-chunks per batch
NKT = S // P                   # 16 k-tiles per batch
VW = 2 * DK + 2                # 130: [vA | 1 | vB | 1]

# estimated PE cost (ns) of one matmul row-block, for filler pacing
MM_NS = 213          # N=512 matmul
TR_NS = 53           # 128-row bf16 transpose
STEP_PE = 854        # scores pair + attnv pair
ACT_NS = 1038        # exp on [128,1024]

# ---------------------------------------------------------------------------
# walrus in this toolchain encodes at most ONE sync wait per instruction
# (two on EventSemaphore).  Tile emits more.  Legalize by carrying excess
# waits on same-engine NOPs inserted right before the instruction (engines
# execute in order, so this is equivalent), and by splitting the kernel-tail
# drain's global-clock waits across a chain of drains.
# ---------------------------------------------------------------------------
_split_counter = [0]


def _legalize_waits(nc):
    inserted = 0
    for fn in nc.m.functions:
        for bb in fn.blocks:
            new_insts = []
            changed = False
            for inst in bb.instructions:
                si = inst.sync_info
                waits = list(si.on_wait) if si is not None and si.on_wait else []
                cap = 2 if inst.opcode == "EventSemaphore" else 1
                if len(waits) > cap:
                    excess, keep = waits[:-cap], waits[-cap:]
                    for w in excess:
                        _split_counter[0] += 1
                        nop = mybir.InstNoOp(
                            name=f"I-waitsplit-{_split_counter[0]}", ins=[], outs=[]
                        )
                        nop.engine = inst.engine
                        nop.sync_info = mybir.SyncInfo(on_wait=[w], on_update=[])
                        new_insts.append(nop)
                        inserted += 1
                    si.on_wait = keep
                    changed = True
                new_insts.append(inst)
            if changed:
                bb.instructions.clear()
                for i in new_insts:
                    bb.instructions.append(i)
    return inserted


class _TC(tile.TileContext):
    def _drain_and_barrier(self, tick_clock, wait_clock):
        drain_inst = self.nc.sync.drain()
        wait_clock.add_sem_waits(
            drain_inst.ins, ScopedClock({None: tick_clock.global_clock})
        )
        si = drain_inst.ins.sync_info
        waits = list(si.on_wait or []) if si is not None else []
        if len(waits) > 1:
            si.on_wait = [waits[0]]
            for w in waits[1:]:
                d = self.nc.sync.drain()
                dsi = d.ins.sync_info
                if dsi is None:
                    d.ins.sync_info = mybir.SyncInfo(on_wait=[w], on_update=[])
                else:
                    dsi.on_wait = [w]
        self.nc.all_engine_barrier()
        assert self.sems is not None
        popped = self.nc._tile_sem_poison_stack.pop()
        assert popped is self._sem_poison
        self.nc.clear_and_free_semaphores(list(self.sems.allocated().values()))
        self.nc.all_engine_barrier()


# ---------------------------------------------------------------------------
# device program (identical on all 8 cores; only input data differs)
# ---------------------------------------------------------------------------
def _build_nc(repeat=1):
    nc = bass.Bass("TRN2", target_bir_lowering=False, debug=False,
                   num_devices=NCORES)
    xt = nc.dram_tensor("xt", [D, T], F32R, kind="ExternalInput").ap()
    wqm = nc.dram_tensor("wqm", [D, P], F32R, kind="ExternalInput").ap()
    wqb = nc.dram_tensor("wqb", [P, 1], F32, kind="ExternalInput").ap()
    wkm = nc.dram_tensor("wkm", [D, P], F32R, kind="ExternalInput").ap()
    wkb = nc.dram_tensor("wkb", [P, 1], F32, kind="ExternalInput").ap()
    wvm = nc.dram_tensor("wvm", [D, P], F32R, kind="ExternalInput").ap()
    wvb = nc.dram_tensor("wvb", [P, 1], F32, kind="ExternalInput").ap()
    wot = nc.dram_tensor("wo", [P, D], F32R, kind="ExternalInput").ap()
    bot = nc.dram_tensor("bo", [KC, P], F32, kind="ExternalInput").ap()
    yt = nc.dram_tensor("yt", [D, T], F16, kind="ExternalOutput").ap()

    with _TC(nc) as tc, nc.allow_low_precision(
            reason="float32r/bf16 matmuls; f16 output partials"):
        _emit(nc, tc, xt, wqm, wqb, wkm, wkb, wvm, wvb, wot, bot, yt,
              repeat=repeat)
    _legalize_waits(nc)
    return nc


def _emit(nc, tc, xt, wqm, wqb, wkm, wkb, wvm, wvb, wot, bot, yt, repeat=1):
    ctxs = []

    def pool(name, bufs, space="SBUF"):
        p = tc.tile_pool(name=name, bufs=bufs, space=space)
        ctxs.append(p)
        return p.__enter__()

    wpool = pool("w", 1)
    persist = pool("persist", 1)
    xpool = pool("x", 3)
    vscrpool = pool("vscr", 2)
    epool = pool("e", 4)
    orawpool = pool("oraw", 3)
    sabpool = pool("sab", 2)
    rcpool = pool("rc", 2)
    yspool = pool("ys", 4)
    pspool = pool("ps", 2, space="PSUM")   # tags s(2bk)x2 + o(1bk)x2 + p(1bk)x2

    # ---- constants / weights (loaded once) ----
    wq_sb = wpool.tile([P, KC, P], F32R)
    wk_sb = wpool.tile([P, KC, P], F32R)
    wv_sb = wpool.tile([P, KC, P], F32R)
    nc.sync.dma_start(wq_sb[:], wqm.rearrange("(k p) d -> p k d", p=P))
    nc.sync.dma_start(wk_sb[:], wkm.rearrange("(k p) d -> p k d", p=P))
    nc.sync.dma_start(wv_sb[:], wvm.rearrange("(k p) d -> p k d", p=P))
    wqb_sb = wpool.tile([P, 1], F32)
    wkb_sb = wpool.tile([P, 1], F32)
    wvb_sb = wpool.tile([P, 1], F32)
    nc.sync.dma_start(wqb_sb[:], wqb[:, :])
    nc.sync.dma_start(wkb_sb[:], wkb[:, :])
    nc.sync.dma_start(wvb_sb[:], wvb[:, :])
    wo_sb = wpool.tile([P, D], F32R)
    nc.sync.dma_start(wo_sb[:], wot[:, :])
    bo_sb = wpool.tile([P, KC], F32)
    nc.sync.dma_start(bo_sb[:], bot.rearrange("m p -> p m"))

    ones_f32 = wpool.tile([P, DK], F32)
    nc.vector.memset(ones_f32[:], 1.0)
    onesDK = wpool.tile([1, DK], F32R)     # lhsT for partition-broadcast mm
    nc.vector.tensor_copy(onesDK[:], ones_f32[0:1, :])
    ident_f32 = wpool.tile([P, P], F32)
    make_identity(nc, ident_f32[:])
    ident_bf = wpool.tile([P, P], BF16)
    nc.vector.tensor_copy(ident_bf[:], ident_f32[:])

    # ---- persistent activations ----
    qT = persist.tile([P, T], F32R)
    kT = persist.tile([P, T], F32R)
    v_comb = persist.tile([P, T // P, VW], BF16)   # [tok%128, tile, 130]
    nc.vector.tensor_copy(
        v_comb[:, :, DK:DK + 1],
        ones_f32[:, 0:1].broadcast_to([P, T // P, 1]))
    nc.vector.tensor_copy(
        v_comb[:, :, VW - 1:VW],
        ones_f32[:, 0:1].broadcast_to([P, T // P, 1]))

    G = repeat * B                 # global batch count
    NCHUNK = G * NQC               # global x-chunk count (4 per batch)
    xtiles = {}                    # chunk idx -> live x tile

    def emit_xdma(m):
        if m >= NCHUNK or m in xtiles:
            return
        b = (m // NQC) % B
        c = m % NQC
        c0 = b * S + c * TCH
        x_ch = xpool.tile([P, KC, TCH], F32R, tag="x", name=f"x_{m}")
        nc.sync.dma_start(
            x_ch[:], xt[:, c0:c0 + TCH].rearrange("(k p) n -> p k n", p=P))
        xtiles[m] = x_ch

    # ---------------- projection filler units for global batch g ------------
    # Units are (cost_ns, fn, open_after): open_after=True while a PSUM "p"
    # accumulation group is mid-flight, during which NO other "p"-tag tile
    # may be allocated (side bc matmuls are gated on this) — otherwise the
    # 2-slot ring can hand the accumulating bank to a later alloc, producing
    # an engine-queue deadlock.
    def make_p_units(g):
        b = g % B
        units = []

        def mk_chunk(c):
            m = g * NQC + c
            tok0 = b * S + c * TCH
            cell = {}

            def mk_mm(w_sb, kc0, key):
                # a PAIR of contraction steps per unit: halves the unit count
                # so the filler queue drains faster than it fills (a growing
                # backlog lets O(g-2) units overlap A(g) and deadlock the
                # oraw ring)
                def u():
                    if kc0 == 0:
                        cell[key] = pspool.tile([P, TCH], F32, tag="p",
                                                name=f"ps_{key}_{m}")
                    for kc in (kc0, kc0 + 1):
                        nc.tensor.matmul(cell[key][:], w_sb[:, kc, :],
                                         xtiles[m][:, kc, :],
                                         start=(kc == 0), stop=(kc == KC - 1))
                return u

            def drain_q():
                nc.vector.tensor_scalar_add(
                    qT[:, tok0:tok0 + TCH], cell['q'][:], wqb_sb[:, 0:1])

            def drain_k():
                nc.vector.tensor_scalar_add(
                    kT[:, tok0:tok0 + TCH], cell['k'][:], wkb_sb[:, 0:1])

            def drain_v():
                v_scr = vscrpool.tile([P, TCH], BF16, tag="vscr",
                                      name=f"vscr_{m}")
                nc.vector.tensor_scalar_add(v_scr[:], cell['v'][:],
                                            wvb_sb[:, 0:1])
                cell['vs'] = v_scr

            def mk_tr(tt):
                def u():
                    vt = tok0 // P + tt
                    tr = pspool.tile([P, 2 * TCH], BF16, tag="p",
                                     name=f"tr_{m}_{tt}")
                    nc.tensor.transpose(tr[:, 0:P],
                                        cell['vs'][:, tt * P:(tt + 1) * P],
                                        ident_bf[:])
                    nc.vector.tensor_copy(v_comb[:, vt, 0:DK], tr[:, 0:DK])
                    nc.vector.tensor_copy(v_comb[:, vt, DK + 1:VW - 1],
                                          tr[:, DK:2 * DK])
                return u

            cunits = []
            for key, w_sb, drain in (('q', wq_sb, drain_q),
                                     ('k', wk_sb, drain_k),
                                     ('v', wv_sb, drain_v)):
                for kc0 in range(0, KC, 2):
                    mm = mk_mm(w_sb, kc0, key)
                    if kc0 == KC - 2:
                        def mmd(mm=mm, drain=drain):
                            mm()
                            drain()
                        cunits.append((2 * MM_NS, mmd, False, 'p'))
                    else:
                        cunits.append((2 * MM_NS, mm, True, 'p'))
            for tt in range(TCH // P):
                cunits.append((TR_NS, mk_tr(tt), False, 'p'))
            # x-DMA prefetch: chunk m+2 issued at the end of chunk m's block
            cunits.append((0, lambda m=m: emit_xdma(m + 2), False, 'p'))
            return cunits

        for c in range(NQC):
            units.extend(mk_chunk(c))
        return units

    # ---------------- out-projection units for (g, qc) ----------------------
    def make_o_units(g, qc, oraw):
        b = g % B
        t0 = b * S
        units = []

        def mk(m):
            def u():
                ps_y = pspool.tile([P, TCH], F32, tag="p", name=f"ps_y_{g}_{qc}_{m}")
                nc.tensor.matmul(ps_y[:], wo_sb[:, m * P:(m + 1) * P],
                                 oraw[:, qc * TCH:(qc + 1) * TCH],
                                 start=True, stop=True)
                ys = yspool.tile([P, TCH], F16, tag="ys", name=f"ys_{g}_{qc}_{m}")
                # bias-add + f16 convert on the Scalar engine (it has slack;
                # 2-byte output is its fast path) to keep DVE off the drain
                nc.scalar.add(ys[:], ps_y[:], bo_sb[:, m:m + 1])
                nc.sync.dma_start(
                    yt[m * P:(m + 1) * P, t0 + qc * TCH:t0 + (qc + 1) * TCH],
                    ys[:])
            return u

        for m in range(KC):
            units.append((MM_NS, mk(m), False, 'o'))
        return units

    # ---------------- attention + interleaving main loop --------------------
    from collections import deque
    filler = deque()
    sides = deque()                # (due_step, fn) — monotone due order
    pend = deque()                 # (qc_key, kc, fn) pending attnv closures
    step_idx = [0]
    ostate = {}                    # qc_key -> dict(oA,oB,sA,sB,rA,rB,oraw,g,qc)

    def emit_attnv(ent):
        qkey, kc, fn = ent
        fn()
        if kc == NKT - 1:
            st = ostate[qkey]

            def drains(st=st):
                sA = sabpool.tile([DK + 1, TCH], F32R, tag="sa",
                                  name=f"sa_{qkey}")
                nc.vector.tensor_copy(sA[:], st['oA'][:])
                sB = sabpool.tile([DK + 1, TCH], F32R, tag="sb",
                                  name=f"sb_{qkey}")
                nc.vector.tensor_copy(sB[:], st['oB'][:])
                st['sA'], st['sB'] = sA, sB
            drains()
            s0 = step_idx[0]

            def recipA():
                rA = rcpool.tile([1, TCH], F32R, tag="rc", name=f"rA_{qkey}")
                nc.vector.reciprocal(rA[:], st['sA'][DK:DK + 1, :])
                st['rA'] = rA

            def recipB():
                rB = rcpool.tile([1, TCH], F32R, tag="rc", name=f"rB_{qkey}")
                nc.vector.reciprocal(rB[:], st['sB'][DK:DK + 1, :])
                st['rB'] = rB

            # bc matmul + its mult consumer MUST be emitted adjacently:
            # a "p"-ring tile whose consumer lands in the queues several
            # steps later lets intervening p-allocs slot-wait on a future
            # DVE instruction -> engine-queue deadlock.
            def bcmultA():
                bca = pspool.tile([DK, TCH], F32, tag="p", name=f"bcA_{qkey}")
                nc.tensor.matmul(bca[:], onesDK[0:1, :], st['rA'][:],
                                 start=True, stop=True)
                sl = slice(st['qc'] * TCH, (st['qc'] + 1) * TCH)
                nc.vector.tensor_mul(st['oraw'][0:DK, sl], st['sA'][0:DK, :],
                                     bca[:, :])

            def bcmultB():
                bcb = pspool.tile([DK, TCH], F32, tag="p", name=f"bcB_{qkey}")
                nc.tensor.matmul(bcb[:], onesDK[0:1, :], st['rB'][:],
                                 start=True, stop=True)
                sl = slice(st['qc'] * TCH, (st['qc'] + 1) * TCH)
                nc.vector.tensor_mul(st['oraw'][DK:P, sl], st['sB'][0:DK, :],
                                     bcb[:, :])

            def unlock():
                filler.extend(make_o_units(st['g'], st['qc'], st['oraw']))

            sides.append((s0 + 1, recipA))
            sides.append((s0 + 2, recipB))
            sides.append((s0 + 3, bcmultA))
            sides.append((s0 + 4, bcmultB))
            sides.append((s0 + 5, unlock))

    # head: x DMAs for batch 0 + raw projections for batch 0
    for m in range(3):
        emit_xdma(m)
    for cost, fn, _open, _k in make_p_units(0):
        fn()

    debt = [0.0]
    p_open = [False]               # a PSUM "p" accumulation group is open

    def do_step(g, qc, kc, target):
        qkey = (g, qc)
        b = g % B
        t0 = b * S
        q0 = t0 + qc * TCH
        kt0 = t0 + kc * P
        s_t = pspool.tile([P, 2 * TCH], F32, tag="s", name=f"s_{g}_{qc}_{kc}")
        nc.tensor.matmul(s_t[:, 0:TCH], kT[0:DK, kt0:kt0 + P],
                         qT[0:DK, q0:q0 + TCH], start=True, stop=True)
        nc.tensor.matmul(s_t[:, TCH:2 * TCH], kT[DK:P, kt0:kt0 + P],
                         qT[DK:P, q0:q0 + TCH], start=True, stop=True)
        e_t = epool.tile([P, 2 * TCH], BF16, tag="e", name=f"e_{g}_{qc}_{kc}")
        nc.scalar.activation(e_t[:], s_t[:], EXPF, scale=0.125)

        def attnv(qkey=qkey, kc=kc, e_t=e_t):
            st = ostate[qkey]
            if kc == 0:
                st['oA'] = pspool.tile([DK + 1, TCH], F32, tag="o",
                                       name=f"oA_{qkey}")
                st['oB'] = pspool.tile([DK + 1, TCH], F32, tag="o",
                                       name=f"oB_{qkey}")
            vt = (qkey[0] % B) * S // P + kc
            nc.tensor.matmul(st['oA'][:], v_comb[:, vt, 0:DK + 1],
                             e_t[:, 0:TCH],
                             start=(kc == 0), stop=(kc == NKT - 1))
            nc.tensor.matmul(st['oB'][:], v_comb[:, vt, DK + 1:VW],
                             e_t[:, TCH:2 * TCH],
                             start=(kc == 0), stop=(kc == NKT - 1))
        pend.append((qkey, kc, attnv))

        # fillers: ALWAYS emit one per step if available (uniform PE fill is
        # what keeps the tensor engine's p-state up — a step with no filler
        # leaves a gap that drops the PE clock 2x); a second is debt-paced.
        # At most one O-unit per step (its ys bias-add runs on the Scalar
        # engine, which must stay below the exp stream's slack).
        if not filler and p_next[0] < G:
            filler.extend(make_p_units(p_next[0]))
            p_next[0] += 1
        debt[0] += target - STEP_PE
        nfill = 0
        n_o = 0
        while filler and nfill < 2 and (nfill == 0
                                        or debt[0] >= filler[0][0] - 1e-9
                                        or len(filler) > 90):
            if filler[0][3] == 'o' and n_o >= 1:
                break
            cost, fn, opn, knd = filler.popleft()
            fn()
            p_open[0] = opn
            debt[0] -= cost
            nfill += 1
            if knd == 'o':
                n_o += 1
        if debt[0] > 2000:
            debt[0] = 2000

        # due side ops (norm pipeline for a finished q-chunk); the bc side
        # matmuls allocate "p"-ring tiles, so they must wait until no
        # projection accumulation group is open
        while sides and sides[0][0] <= step_idx[0] and not p_open[0]:
            _, fn = sides.popleft()
            fn()

        # software-pipelined attn@v (depth 2)
        if len(pend) > 2:
            emit_attnv(pend.popleft())
        step_idx[0] += 1

    p_next = [1]
    for g in range(G):
        b = g % B
        # stage next batch's projections as filler (unless pulled early)
        if p_next[0] <= g + 1 and g + 1 < G:
            filler.extend(make_p_units(g + 1))
            p_next[0] = g + 2
        oraw = orawpool.tile([P, S], F32R, tag="oraw", name=f"oraw_{g}")
        for qc in range(NQC):
            ostate[(g, qc)] = {'g': g, 'qc': qc, 'oraw': oraw}
        supply = sum(c for c, _, _, _ in filler) + 3 * KC * MM_NS
        target = STEP_PE + max(100.0, min(426.0, supply / (NQC * NKT)))
        for qc in range(NQC):
            for kc in range(NKT):
                do_step(g, qc, kc, target)

    # tail: flush pipeline, norm ops, remaining fillers.  Sides may allocate
    # "p"-ring tiles, so fully drain the filler queue (closing any open
    # accumulation group) before each side op.
    while pend:
        emit_attnv(pend.popleft())
        step_idx[0] += 1
    while sides or filler:
        while filler:
            _, fn, _opn, _k = filler.popleft()
            fn()
        if sides:
            _, fn = sides.popleft()
            fn()
            step_idx[0] += 1

    for p in reversed(ctxs):
        p.__exit__(None, None, None)


_CACHED = {}


def _get_nc(repeat=1):
    if repeat not in _CACHED:
        _CACHED[repeat] = _build_nc(repeat=repeat)
    return _CACHED[repeat]


def _make_in_maps(x, wq, bq, wk, bk, wv, bv, wo, bo):
    x = np.asarray(x, np.float32)
    wq, bq = np.asarray(wq, np.float32), np.asarray(bq, np.float32)
    wk, bk = np.asarray(wk, np.float32), np.asarray(bk, np.float32)
    wv, bv = np.asarray(wv, np.float32), np.asarray(bv, np.float32)
    wo, bo = np.asarray(wo, np.float32), np.asarray(bo, np.float32)
    xT = np.ascontiguousarray(x.reshape(T, D).T)
    maps = []
    for c in range(NCORES):
        sl = slice(c * P, (c + 1) * P)
        maps.append({
            "xt": xT,
            "wqm": np.ascontiguousarray(wq[:, sl]),
            "wqb": np.ascontiguousarray(bq[sl])[:, None],
            "wkm": np.ascontiguousarray(wk[:, sl]),
            "wkb": np.ascontiguousarray(bk[sl])[:, None],
            "wvm": np.ascontiguousarray(wv[:, sl]),
            "wvb": np.ascontiguousarray(bv[sl])[:, None],
            "wo": np.ascontiguousarray(wo[sl, :]),
            "bo": (bo if c == 0 else np.zeros_like(bo)).reshape(KC, P).copy(),
        })
    return maps


def kernel(x, wq, bq, wk, bk, wv, bv, wo, bo):
    nc = _get_nc()
    in_maps = _make_in_maps(x, wq, bq, wk, bk, wv, bv, wo, bo)
    res = run_bass_kernel_spmd(nc, in_maps, core_ids=list(range(NCORES)),
                               trace=False)
    yT = np.zeros((D, T), np.float32)
    for c in range(NCORES):
        yT += res.results[c]["yt"].astype(np.float32)
    return np.ascontiguousarray(yT.T).reshape(B, S, D)
